# revision 12
# baseline (speedup 1.0000x reference)
"""Trainium2 Bass kernel for DeformablePatchSampler2d (v5).

out[n, m, c, i, j] = bilinear_sample(x[n, c], row=RY[m, j], col=CX[m, i])

Sampling grid is batch/channel-invariant and known on the host from
`offset`; windows/weights are baked in at build time. Data-parallel over
batch N=8 across 8 cores.

v5 structure (per core):
  - 4 band-PAIRS: partition half s holds band b = p + 4*s (64 channels
    each), so every compute op runs 128 partitions wide.
  - x is pre-cropped AND pre-cast to fp16 on the host into
    x_pack[band, c, 20*span]: per-band row/col windows, contiguous
    ~10.6KB per (band, channel). The DMA fabric runs ~22GB/s/engine at
    that packet size vs ~10 at 500B, and fp16 halves the bytes (the
    rel-err gate is 2e-2; fp16 sampling lands ~5e-4).
  - patch column anchors are spaced exactly 35 px, so slot origins are
    uniform (base + 35*mh, rows rho0=0); per-patch floor jitter is
    absorbed as tap shifts with zero-padded weights. Stage 1 is then
    ONE merged tensor_tensor per row-tap (slot dim stride 35 into the
    band tile) -- DVE op cost is ~flat in elements, so op count rules.
  - stage 2 (col taps) is merged across slots and runs on Pool (its
    bcast-outer mult is 2x faster than DVE's); tap adds stay on DVE.
  - outputs are written [band, c, slot, 16*16] fp16 so each store
    descriptor is 4KB; the host unpermutes, transposes j/i, upcasts.
"""
import numpy as np

_P = 16
_NPH = _NPW = 8
_M = 64
_H = _W = 384
_C = 64
_N = 8
_RW = 20            # rows per band tile
_Q = 20             # stage-1 window width (cols read per slot)
_RT_MAX = 4         # row-tap slots in the weight layout
_CT_MAX = 4         # col-tap slots
_STRIDE = 35        # exact anchor spacing of patch columns
_WSLOT = _RT_MAX * 16 + _CT_MAX * 16   # 128 weight floats per slot
_WPAIR = 8 * _WSLOT                    # 1024 per pair


def _precompute(offset: np.ndarray):
    """Window origins + 3-tap weights, f32 coord math mirroring the reference."""
    offset = offset.astype(np.float32)
    one, half = np.float32(1.0), np.float32(0.5)
    ch = np.linspace(0.0, float(_H), _NPH + 4).astype(np.float32)[2:-2]
    cw = np.linspace(0.0, float(_W), _NPW + 4).astype(np.float32)[2:-2]
    rel = np.arange(_P, dtype=np.float32) - np.float32(_P // 2)
    a = np.arange(_M) // _NPW
    b = np.arange(_M) % _NPW
    hc = ch[a][:, None] + rel[None, :]
    wcen = cw[b][:, None] + rel[None, :]
    gx = (np.float32(2.0) * hc / np.float32(_H - 1) - one) + offset[:, 0:1]
    gy = (np.float32(2.0) * wcen / np.float32(_W - 1) - one) + offset[:, 1:2]
    CX = (((gx + one) * np.float32(_W) - one) * half).astype(np.float64)  # (M,16) cols, dim i
    RY = (((gy + one) * np.float32(_H) - one) * half).astype(np.float64)  # (M,16) rows, dim j

    r0 = np.floor(RY[:, 0]).astype(np.int64)
    c0 = np.floor(CX[:, 0]).astype(np.int64)
    t_r = RY - (r0[:, None] + np.arange(_P)[None, :])
    t_c = CX - (c0[:, None] + np.arange(_P)[None, :])
    assert (t_r >= 0).all() and (t_r < 2).all()
    assert (t_c >= 0).all() and (t_c < 2).all()
    assert r0.min() >= 0 and (r0 + 17).max() <= _H - 1
    assert c0.min() >= 0 and (c0 + 17).max() <= _W - 1

    def taps(t):
        w0 = np.maximum(0.0, 1.0 - t)
        w2 = np.maximum(0.0, t - 1.0)
        return np.stack([w0, 1.0 - w0 - w2, w2], axis=-1).astype(np.float32)

    wr = taps(t_r)  # (M, 16, 3) applies to j (rows)
    wc = taps(t_c)  # (M, 16, 3) applies to i (cols)
    nt_r = np.where(np.abs(wr[:, :, 2]).max(axis=1) > 0, 3, 2)
    nt_c = np.where(np.abs(wc[:, :, 2]).max(axis=1) > 0, 3, 2)
    return r0, c0, wr, wc, nt_r, nt_c


def _plan(offset: np.ndarray):
    r0, c0, wr, wc, nt_r, nt_c = _precompute(offset)
    mw_of = np.arange(_M) % _NPW
    mh_of = np.arange(_M) // _NPW
    band_r0 = np.array([r0[mw_of == b].min() for b in range(8)])
    # uniform slot origins: window of slot mh starts at lo_b + 35*mh
    band_c0 = np.array([(c0 - _STRIDE * mh_of)[mw_of == b].min() for b in range(8)])
    span = int(max(_STRIDE * 7 + _Q,
                   (c0 + 18 - band_c0[mw_of] - 0)[np.arange(_M)].max()))
    span = (span + 1) & ~1
    assert all(r0[m] - band_r0[mw_of[m]] <= 1 for m in range(_M))
    assert band_r0.max() + _RW <= _H
    assert band_c0.min() >= 0 and (band_c0 + span).max() <= _W

    w_all = np.zeros((128, 4 * _WPAIR), dtype=np.float32)
    pairs = []
    for p in range(4):
        bands = (p, p + 4)
        rt_pair, ct_pair = 0, 0
        for mh in range(8):
            for s in range(2):
                m = mh * 8 + bands[s]
                rshift = int(r0[m] - band_r0[bands[s]])
                cshift = int(c0[m] - band_c0[bands[s]] - _STRIDE * mh)
                assert 0 <= rshift <= 1 and 0 <= cshift <= _CT_MAX - 2, \
                    (p, mh, s, rshift, cshift)
                rt_pair = max(rt_pair, rshift + int(nt_r[m]))
                ct_pair = max(ct_pair, cshift + int(nt_c[m]))
                assert cshift + 17 <= _Q and rshift + 17 <= _RW - 1
                base = p * _WPAIR + mh * _WSLOT
                rows = slice(s * 64, (s + 1) * 64)
                wrs = np.zeros((_RT_MAX, 16), dtype=np.float32)
                wcs = np.zeros((_CT_MAX, 16), dtype=np.float32)
                wrs[rshift:rshift + 3] = wr[m].T
                wcs[cshift:cshift + 3] = wc[m].T
                w_all[rows, base:base + _RT_MAX * 16] = wrs.reshape(-1)[None, :]
                w_all[rows, base + _RT_MAX * 16:base + _WSLOT] = \
                    wcs.reshape(-1)[None, :]
        assert rt_pair <= _RT_MAX and ct_pair <= _CT_MAX
        pairs.append(dict(p=p, bands=bands, rt=rt_pair, ct=ct_pair,
                          r0=[int(band_r0[b]) for b in bands],
                          c0=[int(band_c0[b]) for b in bands]))
    return pairs, span, w_all


def _build(pairs, span):
    import concourse.bacc as bacc
    import concourse.mybir as mybir
    from concourse.bass import AP
    from concourse.tile import TileContext

    f16 = mybir.dt.float16
    mult = mybir.AluOpType.mult
    add = mybir.AluOpType.add

    ROWLEN = _RW * span
    TS = 16 * _Q       # t elements per slot (320)

    nc = bacc.Bacc("TRN2", target_bir_lowering=False)
    x_p = nc.dram_tensor("x_pack", (8, _C, ROWLEN), f16, kind="ExternalInput")
    w_d = nc.dram_tensor("w_all", (128, 4 * _WPAIR), f16, kind="ExternalInput")
    out_d = nc.dram_tensor("out_d", (8, _C, 8, 256), f16, kind="ExternalOutput")

    def sub_ap(base_ap, extra_off, free_dims):
        return AP(base_ap.tensor, base_ap.offset + extra_off,
                  [list(base_ap.ap[0])] + [list(d) for d in free_dims])

    with TileContext(nc) as tc:
        with tc.tile_pool(name="fpool", bufs=3) as fpool, \
             tc.tile_pool(name="wpool", bufs=1) as wpool, \
             tc.tile_pool(name="tpool", bufs=2) as tpool, \
             tc.tile_pool(name="mpool", bufs=2) as mpool, \
             tc.tile_pool(name="opool", bufs=2) as opool, \
             tc.tile_pool(name="npool", bufs=2) as npool:
            W_sb = wpool.tile([128, 4 * _WPAIR], f16)
            nc.scalar.dma_start(out=W_sb[:], in_=w_d[:])
            wb = W_sb[:]

            def emit_load(pair):
                F = fpool.tile([128, ROWLEN], f16)
                for s in range(2):
                    src = AP(x_p[:].tensor, pair["bands"][s] * _C * ROWLEN,
                             [[ROWLEN, _C], [1, ROWLEN]])
                    nc.sync.dma_start(out=F[s * 64:(s + 1) * 64, :], in_=src)
                return F

            def emit_store(pair, O):
                for s in range(2):
                    b = pair["bands"][s]
                    dst = AP(out_d[:].tensor, b * (_C * 8 * 256),
                             [[8 * 256, _C], [1, 8 * 256]])
                    nc.scalar.dma_start(out=dst, in_=O[s * 64:(s + 1) * 64, :])

            PREFETCH = 3
            ftiles = {i: emit_load(pairs[i]) for i in range(PREFETCH)}
            pending_store = None
            for pi, pair in enumerate(pairs):
                p = pair["p"]
                bb = ftiles.pop(pi)[:]
                if pi + PREFETCH < len(pairs):
                    ftiles[pi + PREFETCH] = emit_load(pairs[pi + PREFETCH])

                wpair = p * _WPAIR
                T = tpool.tile([128, 8 * TS], f16)
                tb = T[:]
                # stage 1: one merged op per row tap; slot dim strides 35
                # into the band tile, tap k shifts the base row
                for k in range(pair["rt"]):
                    dstt = tb
                    if k > 0:
                        Mt = mpool.tile([128, 8 * TS], f16, name=f"Mt{k}")
                        dstt = Mt[:]
                    dst = sub_ap(dstt, 0, [[TS, 8], [_Q, 16], [1, _Q]])
                    src = sub_ap(bb, k * span, [[_STRIDE, 8], [span, 16], [1, _Q]])
                    w_ap = sub_ap(wb, wpair + k * 16,
                                  [[_WSLOT, 8], [1, 16], [0, _Q]])
                    nc.vector.tensor_tensor(out=dst, in0=src, in1=w_ap, op=mult)
                    if k > 0:
                        nc.vector.tensor_tensor(
                            out=sub_ap(tb, 0, [[1, 8 * TS]]),
                            in0=sub_ap(tb, 0, [[1, 8 * TS]]),
                            in1=sub_ap(Mt[:], 0, [[1, 8 * TS]]),
                            op=add)
                # stage 2: merged col-tap mults on Pool, adds on DVE
                O = opool.tile([128, 8 * 256], f16)
                ob = O[:]
                for ik in range(pair["ct"]):
                    w_ap = sub_ap(wb, wpair + _RT_MAX * 16 + ik * 16,
                                  [[_WSLOT, 8], [0, 16], [1, 16]])
                    in0 = sub_ap(tb, ik, [[TS, 8], [_Q, 16], [1, 16]])
                    if ik == 0:
                        o_ap = sub_ap(ob, 0, [[256, 8], [16, 16], [1, 16]])
                        nc.gpsimd.tensor_tensor(out=o_ap, in0=in0, in1=w_ap,
                                                op=mult)
                    else:
                        MO = npool.tile([128, 8 * 256], f16, name=f"MO{ik}")
                        m_ap = sub_ap(MO[:], 0, [[256, 8], [16, 16], [1, 16]])
                        nc.gpsimd.tensor_tensor(out=m_ap, in0=in0, in1=w_ap,
                                                op=mult)
                        nc.vector.tensor_tensor(
                            out=sub_ap(ob, 0, [[1, 8 * 256]]),
                            in0=sub_ap(ob, 0, [[1, 8 * 256]]),
                            in1=sub_ap(MO[:], 0, [[1, 8 * 256]]),
                            op=add)
                if pending_store is not None:
                    emit_store(*pending_store)
                pending_store = (pair, O)

            emit_store(*pending_store)
    nc.compile()
    return nc


def _prepare(offset):
    pairs, span, w_all = _plan(offset)
    nc = _build(pairs, span)
    aux = dict(pairs=pairs, span=span, w_all=w_all.astype(np.float16))
    return nc, aux


def _pack_x(xn, aux):
    """(C, H, W) f32 -> (8, C, 20*span) fp16 band crops."""
    span = aux["span"]
    out = np.empty((8, _C, _RW * span), dtype=np.float16)
    for p in aux["pairs"]:
        for s in range(2):
            b = p["bands"][s]
            r0, c0 = p["r0"][s], p["c0"][s]
            out[b] = xn[:, r0:r0 + _RW, c0:c0 + span].astype(
                np.float16).reshape(_C, -1)
    return out


def _run(nc, x, aux, **kwargs):
    from concourse.bass_utils import run_bass_kernel_spmd
    in_maps = [{"x_pack": _pack_x(x[n], aux), "w_all": aux["w_all"]}
               for n in range(_N)]
    return run_bass_kernel_spmd(nc, in_maps, core_ids=list(range(_N)), **kwargs)


def _postprocess(out_dev, pairs):
    """out_dev: (8 bands, C, 8 slots, 256) fp16 -> (M, C, 16, 16) f32.

    Device slot layout is [j][i]-major (keeps DVE last dims packed);
    semantic output is [i][j]."""
    out = np.empty((_M, _C, _P, _P), dtype=np.float32)
    for b in range(8):
        for mh in range(8):
            out[mh * 8 + b] = out_dev[b, :, mh].astype(
                np.float32).reshape(_C, _P, _P).transpose(0, 2, 1)
    return out


def kernel(x: np.ndarray, offset: np.ndarray) -> np.ndarray:
    x = np.asarray(x, dtype=np.float32)
    offset = np.asarray(offset, dtype=np.float32)
    nc, aux = _prepare(offset)
    res = _run(nc, x, aux)
    return np.stack([_postprocess(res.results[n]["out_d"], aux["pairs"])
                     for n in range(_N)])
